# revision 33
# baseline (speedup 1.0000x reference)
"""Trainium2 Bass kernel for DyadicCrossAttention.

Sharding: 8 cores = 2 batches x 4 head-groups (2 heads of d=32 each, i.e. a
64-channel slice of HID=256). Each core computes its batch's q/k/v projections
for its 64 channels, both cross-attention directions for its 2 heads, and a
partial output projection Wo[:, ch] @ out[ch]. The host sums the 4 per-batch
partials and adds the constant bias vectors (bo + Wo @ bv, both independent of
the pixel, since softmax weights sum to 1).

Matmul operands are fp16 (fp32 PSUM accumulation). The softmax exp is the
bottleneck; it is split across TWO engines:
 - the scalar (ACT) engine runs activation(Exp, scale=16) at 1 col/cycle,
 - the vector (DVE) engine runs a pair of custom microprogrammed ops
   (EXP16A: clamped relative-minimax cubic of e^r, EXP16B: ^16 + scale) on a
   ~20% slice of the score tiles, at ~2.2x the ACT per-tile cost but in
   parallel with it. SCALE/16 is folded into Wq/bq host-side so PSUM scores
   are r = logit/16 in [-0.30, 0.425]; composite rel error ~1.4e-3.

Direction order: dir-2 (q2 k1 -> v1) runs FIRST because its keys come from
modal-1 (half the DMA bytes of modal-2), so the exp stream starts ~3us in
while x2 is still loading. Dir-1 follows and ends on the small 256-col block
to shorten the drain tail. Scores are transposed (keys on partitions);
softmax denominators ride along as ones-columns in the attn*V matmul, and
per-block epilogues (normalize + Wo projection) are deferred into the next
block so the in-order engines never interpose long work before the next
exp's input is ready.
"""

import sys

import numpy as np

sys.path.insert(0, "/opt/trn_rl_repo")

B, DIM1, DIM2, HID, HEADS, H, W, OUT = 2, 256, 512, 256, 8, 48, 48, 256
HD = HID // HEADS  # 32
SCALE = float(HD) ** -0.5
N = H * W  # 2304
NJ = N // 128  # 18 key chunks
NTILES = [(0, 256), (256, 512), (768, 512), (1280, 512), (1792, 512)]
NQUARTERS = [(0, 576), (576, 576), (1152, 576), (1728, 576)]

MM_MODE = "f16"        # "f16" or "bf16" matmul operand format
TRACE = False          # set by test.py for profiled runs
TRACE_KWARGS = {}
LAST_RESULTS = None    # BassKernelResults of the last run (for test.py)

# ---- custom DVE exp: out = exp(16*x) in two ops ----
# free minimax cubic for e^r on [-0.30, 0.425], normalized to P(0)=1
EXP_C1N = 0.999598597084961
EXP_C2N = 0.5050122210021222
EXP_C3N = 0.17626501373112596
EXP_PLO = 0.7408123653939622   # P(-0.30): clamp floor (== clamping r)
EXP_KAPPA = 0.9986888065713158  # c0^16

# j-steps of each processed block where the DVE "assists": head-0's exp
# stays on ACT (half-size instruction) and head-1's scores detour through a
# separate rvp PSUM tile that only the DVE custom-exp reads, so the stp
# double-buffer never waits on the slower DVE path.
# keyed by (dirpos, border_position); blocks are 18 j-steps each
DVE_JS = {}
for _dp in (0, 1):
    for _bp in range(5):
        if (_dp, _bp) in ((0, 0), (1, 4)):   # first-ever and last-ever blocks
            DVE_JS[(_dp, _bp)] = ()
        else:
            DVE_JS[(_dp, _bp)] = (3, 7, 11, 15)

_CACHE = {}


def _register_exp16():
    from concourse import dve_ops
    from concourse.dve_spec import C0, C1, C2, One, Spec, Src0, lower, maxx, sq
    from concourse.dve_spec import _has_src1
    from concourse.dve_uop import DveOpSpec

    have = {op.name: op for op in dve_ops.OPS}
    if "EXP16A_ANT" in have and "EXP16B_ANT" in have:
        return have["EXP16A_ANT"], have["EXP16B_ANT"]

    def ref_a(in0, in1, s0, s1, imm2):
        x = in0.astype(np.float32)
        return ((s0 * x + s1) * (x * x) + (imm2 * x + 1.0)).astype(np.float32)

    def ref_b(in0, in1, s0, s1, imm2):
        p = np.maximum(in0.astype(np.float32), s0)
        for _ in range(4):
            p = (p * p).astype(np.float32)
        return (p * s1).astype(np.float32)

    spec_a = Spec(body=(Src0 * C0 + C1) * sq(Src0) + (Src0 * C2 + One),
                  reference=ref_a)
    spec_b = Spec(body=sq(sq(sq(sq(maxx(Src0, C0))))) * C1, reference=ref_b)

    out = []
    for name, spec in (("EXP16A_ANT", spec_a), ("EXP16B_ANT", spec_b)):
        row = max(dve_ops._SUB_OPCODE_FOR_NAME.values()) + 1
        assert row < 0x20
        shas = {}
        for ver in ("v3", "v4"):
            tmp = DveOpSpec(name=name, opcode=row, uops=lower(spec, ver=ver),
                            rd1_en=_has_src1(spec))
            shas[ver] = tmp.sha(ver)
        op = dve_ops.DveOp(name, spec, subdim=False, uops_sha=shas)
        dve_ops.OPS.append(op)
        dve_ops.CUSTOM_DVE_SPECS[name] = spec
        dve_ops._SUB_OPCODE_FOR_NAME[name] = row
        out.append(op)
    return out


def _split_multiwait(nc, mybir, limit=1):
    """Walrus rejects instructions carrying >limit semaphore waits; move the
    excess onto InstNoOp instructions inserted just before on the same engine."""
    for f in nc.m.functions:
        for bb in f.blocks:
            out = []
            changed = False
            for inst in bb.instructions:
                si = inst.sync_info
                if si is not None and len(si.on_wait) > limit:
                    waits = list(si.on_wait)
                    pre, keep = waits[:-limit], waits[-limit:]
                    for ci in range(0, len(pre), limit):
                        nop = mybir.InstNoOp(
                            name=f"{inst.name}-ws{ci}", ins=[], outs=[]
                        )
                        nop.engine = inst.engine
                        nop.sync_info = mybir.SyncInfo(
                            on_wait=pre[ci : ci + limit], on_update=[]
                        )
                        out.append(nop)
                    inst.sync_info = mybir.SyncInfo(
                        on_wait=keep, on_update=list(si.on_update)
                    )
                    changed = True
                out.append(inst)
            if changed:
                bb.instructions = out


def _build(mm_mode=None, reps=1):
    import concourse.bass as bass
    import concourse.tile as tile
    from concourse import mybir
    from concourse.library_overlay import lower_extended_insts

    if mm_mode is None:
        mm_mode = MM_MODE
    f32 = mybir.dt.float32
    md = {"bf16": mybir.dt.bfloat16, "f16": mybir.dt.float16}[mm_mode]
    AF = mybir.ActivationFunctionType
    op_a, op_b = _register_exp16()

    nc = bass.Bass()

    x1_d = nc.dram_tensor("x1", [128, 2, N], md, kind="ExternalInput")
    x2_d = nc.dram_tensor("x2", [128, 4, N], md, kind="ExternalInput")
    wqk1_d = nc.dram_tensor("wqk1", [128, 2, 128], md, kind="ExternalInput")
    wv1_d = nc.dram_tensor("wv1", [128, 2, 64], md, kind="ExternalInput")
    wqk2_d = nc.dram_tensor("wqk2", [128, 4, 128], md, kind="ExternalInput")
    wv2_d = nc.dram_tensor("wv2", [128, 4, 64], md, kind="ExternalInput")
    wo_d = nc.dram_tensor("wo", [32, 4, OUT], md, kind="ExternalInput")
    bqk1_d = nc.dram_tensor("bqk1", [128, 1], f32, kind="ExternalInput")
    bqk2_d = nc.dram_tensor("bqk2", [128, 1], f32, kind="ExternalInput")
    e33_d = nc.dram_tensor("e33", [33, 64], md, kind="ExternalInput")
    # partition-major output layout: [128 partitions, 2 row-halves, N]
    y1_d = nc.dram_tensor("y1", [128, 2, N], f32, kind="ExternalOutput")
    y2_d = nc.dram_tensor("y2", [128, 2, N], f32, kind="ExternalOutput")

    with tile.TileContext(nc) as tc, nc.allow_low_precision(
        reason="f16 matmul operands by design; fp32 psum accumulation"
    ):
        with tc.tile_pool(name="const", bufs=1) as const:
            # ---- resident tensors / DMAs ----
            # dir-2 runs first, so modal-1 (its key/value source) loads with
            # priority on the sync queue; modal-2 streams on the gpsimd
            # queue (its first chunk early: q2 block 0 gates the first
            # scores).
            wqk1 = const.tile([128, 2, 128], md)
            nc.sync.dma_start(wqk1[:], wqk1_d[:])
            # wqk2 + small early tensors ride the scalar-engine DGE queue so
            # they don't serialize the two x-streams (ACT is idle this early)
            wqk2 = const.tile([128, 4, 128], md)
            nc.scalar.dma_start(wqk2[:], wqk2_d[:])
            x1s = const.tile([128, 2, N], md)
            x2s = const.tile([128, 4, N], md)
            # prologue pieces split per channel-chunk so each projection
            # matmul starts as soon as its own contraction slice lands
            for c in range(2):
                nc.sync.dma_start(x1s[:, c, 0:256], x1_d[:, c, 0:256])
            # dir-2 starts on its 512-col block (cols 256-768): q2 of that
            # ntile is the only x2 piece in the critical DMA window
            for c0 in range(2):
                nc.gpsimd.dma_start(
                    x2s[:, 2 * c0 : 2 * c0 + 2, 256:768],
                    x2_d[:, 2 * c0 : 2 * c0 + 2, 256:768],
                )
            bqk1 = const.tile([128, 1], f32)
            nc.scalar.dma_start(bqk1[:], bqk1_d[:])
            bqk2 = const.tile([128, 1], f32)
            nc.scalar.dma_start(bqk2[:], bqk2_d[:])
            wv1 = const.tile([128, 2, 64], md)
            nc.scalar.dma_start(wv1[:], wv1_d[:])
            wv2 = const.tile([128, 4, 64], md)
            wo = const.tile([32, 4, OUT], md)
            e33 = const.tile([33, 64], md)
            nc.scalar.dma_start(e33[:], e33_d[:])
            # the DMA engines drain ~serially in issue order: x1's remaining
            # ntile chunks (dir-2's key sweep k1 <- x1 paces the first ~20us)
            # interleaved with x2's per-block q2 chunks just ahead of each
            # block's start; x2's tail (v2, q2 of the final small block)
            # lands long before dir-1 consumes it.
            nc.sync.dma_start(x1s[:, :, 256:768], x1_d[:, :, 256:768])
            nc.sync.dma_start(x1s[:, :, 768:1280], x1_d[:, :, 768:1280])
            nc.sync.dma_start(x2s[:, :, 768:1280], x2_d[:, :, 768:1280])
            nc.sync.dma_start(x1s[:, :, 1280:1792], x1_d[:, :, 1280:1792])
            nc.sync.dma_start(x1s[:, :, 1792:2304], x1_d[:, :, 1792:2304])
            nc.sync.dma_start(x2s[:, :, 1280:1792], x2_d[:, :, 1280:1792])
            nc.gpsimd.dma_start(wv2[:], wv2_d[:])
            nc.sync.dma_start(x2s[:, :, 1792:2304], x2_d[:, :, 1792:2304])
            nc.sync.dma_start(x2s[:, :, 0:256], x2_d[:, :, 0:256])
            nc.sync.dma_start(wo[:], wo_d[:])
            wo1a, wo1b = wo[:, 0, :], wo[:, 1, :]
            wo2a, wo2b = wo[:, 2, :], wo[:, 3, :]

            # projections (channel-major q/k; pixel-major v with ones columns)
            q1s = const.tile([64, N], md)
            k1s = const.tile([64, N], md)
            q2s = const.tile([64, N], md)
            k2s = const.tile([64, N], md)
            # per key-chunk j, 128 columns: [v_h0(32) | 1 | pad] [v_h1(32) | 1 | pad]
            v1T = const.tile([128, NJ, 128], md)
            v2T = const.tile([128, NJ, 128], md)
            for vt in (v1T, v2T):
                for ones_col in (vt[:, :, 32:33], vt[:, :, 96:97]):
                    nc.vector.memset(ones_col, 1.0)
            # reciprocal staging: rows 0 and 32 live, the rest stays zero
            rdens = []
            for _ri in range(2):
                rd = const.tile([64, 512], md, name=f"rden{_ri}")
                nc.vector.memset(rd[:], 0.0)
                rdens.append(rd)

            with (
                tc.tile_pool(name="stp", bufs=2, space="PSUM") as stp,
                tc.tile_pool(name="u0p", bufs=2, space="PSUM") as u0p,
                tc.tile_pool(name="rvp", bufs=1, space="PSUM") as rvp,
                tc.tile_pool(name="opp", bufs=1, space="PSUM") as opp,
                tc.tile_pool(name="ptp", bufs=6) as ptp,
                tc.tile_pool(name="midp", bufs=2) as midp,
                tc.tile_pool(name="sbs", bufs=3) as sbs,
            ):
                # ---- deferred projection emission ----
                qk_done = {(i, m): False
                           for i in range(len(NTILES)) for m in (1, 2)}
                v_done = {(m, j): False for m in (1, 2) for j in range(NJ)}

                def emit_qk_m(i, m, pool=None, k_first=False, defer_other=None):
                    if qk_done[(i, m)]:
                        return
                    qk_done[(i, m)] = True
                    n0, nt = NTILES[i]
                    wqk, xs, nch, qdst, kdst, bqk = (
                        (wqk1, x1s, 2, q1s, k1s, bqk1) if m == 1
                        else (wqk2, x2s, 4, q2s, k2s, bqk2)
                    )
                    pool = pool or opp
                    tag = "op" if pool is opp else "rv"
                    ps = pool.tile([128, 512], f32, tag=tag, name=f"qkps{i}_{m}")
                    for c in range(nch):
                        nc.tensor.matmul(
                            ps[:, :nt], wqk[:, c, :], xs[:, c, n0 : n0 + nt],
                            start=(c == 0), stop=(c == nch - 1),
                        )
                    copies = [
                        (qdst, ps[0:64, :nt], bqk[0:64, :]),
                        (kdst, ps[64:128, :nt], bqk[64:128, :]),
                    ]
                    if k_first:
                        copies.reverse()
                    if defer_other is not None:
                        # only the first (needed) half now; queue the other
                        # so it doesn't gate the first scores on the DVE
                        dst, src, b = copies[0]
                        nc.vector.tensor_scalar_add(dst[:, n0 : n0 + nt], src, b)
                        dst, src, b = copies[1]
                        work_q.append((defer_other, (
                            lambda dst=dst, src=src, b=b:
                            nc.vector.tensor_scalar_add(dst[:, n0:n0 + nt], src, b)
                        )))
                        return
                    for dst, src, b in copies:
                        nc.vector.tensor_scalar_add(dst[:, n0 : n0 + nt], src, b)

                def emit_v(mod, j):
                    if v_done[(mod, j)]:
                        return
                    v_done[(mod, j)] = True
                    xs, wv, nch, vdst = (
                        (x1s, wv1, 2, v1T) if mod == 1 else (x2s, wv2, 4, v2T)
                    )
                    pvt = rvp.tile([128, 64], f32, tag="rv", name=f"pv{mod}_{j}")
                    for c in range(nch):
                        nc.tensor.matmul(
                            pvt[:], xs[:, c, j * 128 : (j + 1) * 128], wv[:, c, :],
                            start=(c == 0), stop=(c == nch - 1),
                        )
                    nc.vector.tensor_copy(vdst[:, j, 0:32], pvt[:, 0:32])
                    nc.vector.tensor_copy(vdst[:, j, 64:96], pvt[:, 32:64])

                pending_v = [(1, j) for j in range(NJ)] + [(2, j) for j in range(NJ)]
                pv_idx = [0]

                def trickle_v():
                    while pv_idx[0] < len(pending_v) and v_done[pending_v[pv_idx[0]]]:
                        pv_idx[0] += 1
                    if pv_idx[0] < len(pending_v):
                        mod, j = pending_v[pv_idx[0]]
                        emit_v(mod, j)

                # ---- prologue: unblock the first scores (dir 2: k1^T q2,
                # first processed block is ntile 1) ----
                work_q = []   # (ready_t, closure) FIFO; head runs when ready
                emit_qk_m(0, 1, pool=rvp, k_first=True, defer_other=2)
                emit_qk_m(1, 2, pool=opp, k_first=False, defer_other=3)

                # ---- attention: flat software-pipelined stream ----
                LAG = 2
                items = []
                for dirpos, (qs, ks, vT, woa, wob, ydst, vmod, border) in enumerate((
                    (q2s, k1s, v1T, wo2a, wo2b, y2_d, 1, (1, 2, 3, 4, 0)),
                    (q1s, k2s, v2T, wo1a, wo1b, y1_d, 2, (1, 2, 3, 4, 0)),
                ) * reps):
                    for bpos, bi in enumerate(border):
                        n0, nt = NTILES[bi]
                        dve_js = DVE_JS.get((dirpos % 2, bpos), ())
                        for j in range(NJ):
                            items.append((qs, ks, vT, woa, wob, ydst, vmod,
                                          bi, n0, nt, j, j in dve_js))

                blkc = [0]

                def mk_epi_parts(ut, rden, woa, wob, ydst, n0, nt):
                    """The per-block epilogue, split into three work-queue
                    pieces so no single pop clumps >1us on one engine. The
                    DVE chain is ordered rb0,o0,rb1,o1 so the first output
                    matmul can start after only half of it."""
                    u0t = ut[0:33, :]
                    u1t = ut[64:97, :]
                    o0 = sbs.tile([32, 512], md, tag="o0")
                    o1 = sbs.tile([32, 512], md, tag="o1")

                    def epi_a():
                        rbt = rvp.tile([64, 512], f32, tag="rv", name="rbt")
                        nc.tensor.matmul(rbt[:, :nt], e33[:], rden[0:33, :nt])
                        rb0 = sbs.tile([32, 512], f32, tag="rb0")
                        nc.vector.tensor_copy(rb0[:, :nt], rbt[0:32, :nt])
                        nc.vector.tensor_mul(o0[:, :nt], u0t[0:32, :nt], rb0[:, :nt])
                        rb1 = sbs.tile([32, 512], f32, tag="rb1")
                        nc.vector.tensor_copy(rb1[:, :nt], rbt[32:64, :nt])
                        nc.vector.tensor_mul(o1[:, :nt], u1t[0:32, :nt], rb1[:, :nt])

                    if nt <= 256:
                        # small block (incl. the drain-tail block): both
                        # output halves in one PSUM tile and one DMA
                        def epi_small():
                            opt = opp.tile([128, 2, 256], f32, tag="op")
                            for mt in range(2):
                                nc.tensor.matmul(
                                    opt[:, mt, :nt],
                                    woa[:, mt * 128 : (mt + 1) * 128],
                                    o0[:, :nt], start=True, stop=False,
                                )
                                nc.tensor.matmul(
                                    opt[:, mt, :nt],
                                    wob[:, mt * 128 : (mt + 1) * 128],
                                    o1[:, :nt], start=False, stop=True,
                                )
                            yt = sbs.tile([128, 2, 256], f32, tag="yt")
                            nc.vector.tensor_copy(yt[:, :, :nt], opt[:, :, :nt])
                            nc.sync.dma_start(
                                ydst[:, :, n0 : n0 + nt], yt[:, :, :nt]
                            )
                        def epi_nop():
                            pass
                        return epi_a, epi_small, epi_nop

                    def mk_epi_mt(mt):
                        def epi_mt():
                            opt = opp.tile([128, 512], f32, tag="op")
                            nc.tensor.matmul(
                                opt[:, :nt], woa[:, mt * 128 : (mt + 1) * 128],
                                o0[:, :nt], start=True, stop=False,
                            )
                            nc.tensor.matmul(
                                opt[:, :nt], wob[:, mt * 128 : (mt + 1) * 128],
                                o1[:, :nt], start=False, stop=True,
                            )
                            yt = sbs.tile([128, 512], f32, tag="yt")
                            nc.vector.tensor_copy(yt[:, :nt], opt[:, :nt])
                            nc.sync.dma_start(
                                ydst[:, mt, n0 : n0 + nt], yt[:, :nt]
                            )
                        return epi_mt

                    return epi_a, mk_epi_mt(0), mk_epi_mt(1)

                cur_u = {}    # "live" ut/rden for the consuming block
                pts = {}

                def emit_u(t2, it, t_now):
                    (qs, ks, vT, woa, wob, ydst, vmod, bi, n0, nt, jj, _dv) = it
                    emit_v(vmod, jj)
                    if jj == 0:
                        ut = u0p.tile([128, 512], f32, tag="u0", name="ut")
                        rden = rdens[blkc[0] % 2]
                        blkc[0] += 1
                        cur_u[(id(items), t2 - jj)] = (ut, rden)
                    ut, rden = cur_u[(id(items), t2 - jj)]
                    u0t = ut[0:33, :]
                    u1t = ut[64:97, :]
                    pt = pts.pop(t2)
                    nc.tensor.matmul(
                        u0t[:, :nt], vT[:, jj, 0:33], pt[:, 0, :nt],
                        start=(jj == 0), stop=(jj == NJ - 1),
                    )
                    nc.tensor.matmul(
                        u1t[:, :nt], vT[:, jj, 64:97], pt[:, 1, :nt],
                        start=(jj == 0), stop=(jj == NJ - 1),
                    )
                    if jj == NJ - 1:
                        del cur_u[(id(items), t2 - jj)]
                        nc.vector.reciprocal(rden[0:1, :nt], u0t[32:33, :nt])
                        nc.vector.reciprocal(rden[32:33, :nt], u1t[32:33, :nt])
                        ea, e0, e1 = mk_epi_parts(ut, rden, woa, wob, ydst,
                                                  n0, nt)
                        work_q.append((t_now + 3, ea))
                        work_q.append((t_now + 5, e0))
                        work_q.append((t_now + 7, e1))

                t = 0
                n_items = len(items)
                next_u = [0]
                while (t < n_items or next_u[0] < n_items or work_q
                       or pv_idx[0] < len(pending_v)):
                    if t < n_items:
                        it = items[t]
                        (qs, ks, vT, woa, wob, ydst, vmod, bi, n0, nt, j,
                         dve_exp) = it
                        kmod = vmod                # k comes from the v modal
                        qmod = 3 - vmod
                        emit_qk_m(bi, qmod)
                        if j == 8 and t + NJ < n_items:
                            # pre-emit the next block's q projection mid-block
                            # so it's off the critical path at the boundary
                            nit = items[t + NJ - 8]
                            work_q.append(
                                (t + 1, (lambda nbi=nit[7], nqm=3 - nit[6]:
                                         emit_qk_m(nbi, nqm)))
                            )
                        last_col = 128 * (j + 1) - 1
                        for ni, (tn0, tnt) in enumerate(NTILES):
                            if tn0 <= last_col < tn0 + tnt:
                                if not qk_done[(ni, kmod)]:
                                    emit_qk_m(ni, kmod)
                                    if not qk_done[(ni, qmod)]:
                                        # other modal's projection of this
                                        # ntile: needed later, spread it out
                                        work_q.append(
                                            (t + 1, (lambda ni=ni, qmod=qmod:
                                                     emit_qk_m(ni, qmod)))
                                        )
                                break
                        st = stp.tile([128, 2, 512], f32, tag="st")
                        pt = ptp.tile([128, 2, 512], md, tag="pt")
                        nc.tensor.matmul(
                            st[:, 0, :nt],
                            ks[0:32, j * 128 : (j + 1) * 128],
                            qs[0:32, n0 : n0 + nt],
                        )
                        if dve_exp:
                            # head-1 scores into a decoupled PSUM slot; ACT
                            # only exps head 0 (half-size instruction)
                            rv_t = rvp.tile([128, 512], f32, tag="rv",
                                            name="dve_st")
                            nc.tensor.matmul(
                                rv_t[:, :nt],
                                ks[32:64, j * 128 : (j + 1) * 128],
                                qs[32:64, n0 : n0 + nt],
                            )
                            nc.scalar.activation(
                                pt[:, 0, :nt], st[:, 0, :nt], AF.Exp,
                                scale=16.0,
                            )
                            mid = midp.tile([128, 512], f32, tag="mid")
                            nc.vector._custom_dve(
                                op_a, out=mid[:, :nt], in0=rv_t[:, :nt],
                                s0=EXP_C3N, s1=EXP_C2N, imm2=EXP_C1N,
                            )
                            nc.vector._custom_dve(
                                op_b, out=pt[:, 1, :nt], in0=mid[:, :nt],
                                s0=EXP_PLO, s1=EXP_KAPPA,
                            )
                        else:
                            nc.tensor.matmul(
                                st[:, 1, :nt],
                                ks[32:64, j * 128 : (j + 1) * 128],
                                qs[32:64, n0 : n0 + nt],
                            )
                            nc.scalar.activation(
                                pt[:, :, :nt], st[:, :, :nt], AF.Exp, scale=16.0
                            )
                        pts[t] = pt
                    # attn*V consumption: strictly in tile order (the PSUM
                    # accumulation group's stop matmul must be emitted last),
                    # but per-tile due times let the slower DVE-exp tiles lag
                    # further behind, and spread the first block's tail into
                    # the next block's filler slots.
                    emitted_u = 0
                    while next_u[0] < min(t + 1, n_items) and emitted_u < 2:
                        t2 = next_u[0]
                        it2 = items[t2]
                        jj = it2[10]
                        due = t2 + LAG
                        if t2 < NJ and jj >= 13:
                            due = NJ + 2 + (jj - 13)
                        elif it2[11]:
                            due = t2 + LAG + 3
                        if due > t:
                            break
                        next_u[0] += 1
                        emitted_u += 1
                        emit_u(t2, it2, t)
                    if work_q and work_q[0][0] <= t:
                        work_q.pop(0)[1]()
                    elif pv_idx[0] < len(pending_v):
                        trickle_v()
                    t += 1

    _split_multiwait(nc, mybir, limit=1)
    lower_extended_insts(nc)
    return nc


def _get_nc():
    key = ("nc", MM_MODE)
    if key not in _CACHE:
        _CACHE[key] = _build()
    return _CACHE[key]


def kernel(
    modal1_feat, modal2_feat, Wq1, bq1, Wk1, bk1, Wv1, bv1,
    Wq2, bq2, Wk2, bk2, Wv2, bv2, Wo1, bo1, Wo2, bo2,
):
    global LAST_RESULTS
    from concourse.bass_utils import run_bass_kernel_spmd

    if MM_MODE == "bf16":
        import ml_dtypes
        md_np = ml_dtypes.bfloat16
    else:
        md_np = np.float16

    f = np.float32
    modal1_feat = np.asarray(modal1_feat, f)
    modal2_feat = np.asarray(modal2_feat, f)
    Wq1, bq1 = np.asarray(Wq1, f), np.asarray(bq1, f)
    Wk1, bk1 = np.asarray(Wk1, f), np.asarray(bk1, f)
    Wv1, bv1 = np.asarray(Wv1, f), np.asarray(bv1, f)
    Wq2, bq2 = np.asarray(Wq2, f), np.asarray(bq2, f)
    Wk2, bk2 = np.asarray(Wk2, f), np.asarray(bk2, f)
    Wv2, bv2 = np.asarray(Wv2, f), np.asarray(bv2, f)
    Wo1, bo1 = np.asarray(Wo1, f), np.asarray(bo1, f)
    Wo2, bo2 = np.asarray(Wo2, f), np.asarray(bo2, f)

    # fold SCALE/16 into the q projections: PSUM scores become logit/16
    qs = SCALE / 16.0
    Wq1s, bq1s = Wq1 * qs, bq1 * qs
    Wq2s, bq2s = Wq2 * qs, bq2 * qs

    e33 = np.zeros((33, 64), f)
    e33[0, 0:32] = 1.0
    e33[32, 32:64] = 1.0

    def cvt(a):
        return np.ascontiguousarray(np.asarray(a, md_np))

    def pack(a, nch):
        # [nch*128, cols] -> partition-major [128, nch, cols]
        return a.reshape(nch, 128, -1).transpose(1, 0, 2)

    in_maps = []
    for core in range(8):
        b, hg = core // 4, core % 4
        ch = slice(hg * 64, hg * 64 + 64)
        cha = slice(hg * 64, hg * 64 + 32)
        chb = slice(hg * 64 + 32, hg * 64 + 64)
        in_maps.append({
            "x1": cvt(pack(modal1_feat[b].reshape(DIM1, N), 2)),
            "x2": cvt(pack(modal2_feat[b].reshape(DIM2, N), 4)),
            "wqk1": cvt(pack(np.concatenate([Wq1s[ch].T, Wk1[ch].T], axis=1), 2)),
            "wv1": cvt(pack(Wv1[ch].T, 2)),
            "wqk2": cvt(pack(np.concatenate([Wq2s[ch].T, Wk2[ch].T], axis=1), 4)),
            "wv2": cvt(pack(Wv2[ch].T, 4)),
            "wo": cvt(np.stack([Wo1[:, cha].T, Wo1[:, chb].T,
                                Wo2[:, cha].T, Wo2[:, chb].T], axis=1)),
            "bqk1": np.ascontiguousarray(
                np.concatenate([bq1s[ch], bk1[ch]])[:, None]),
            "bqk2": np.ascontiguousarray(
                np.concatenate([bq2s[ch], bk2[ch]])[:, None]),
            "e33": cvt(e33),
        })

    nc = _get_nc()
    res = run_bass_kernel_spmd(
        nc, in_maps, core_ids=list(range(8)), trace=TRACE, **TRACE_KWARGS
    )
    LAST_RESULTS = res

    out1 = np.zeros((B, OUT, N), f)
    out2 = np.zeros((B, OUT, N), f)
    for core in range(8):
        b = core // 4
        out1[b] += res.results[core]["y1"].transpose(1, 0, 2).reshape(OUT, N)
        out2[b] += res.results[core]["y2"].transpose(1, 0, 2).reshape(OUT, N)
    # constant (per-pixel-independent) bias terms: bo + Wo @ bv
    out1 += (bo1 + Wo1 @ bv2)[None, :, None]
    out2 += (bo2 + Wo2 @ bv1)[None, :, None]
    return (
        out1.reshape(B, OUT, H, W),
        out2.reshape(B, OUT, H, W),
    )


# revision 54
# speedup vs baseline: 1.0022x; 1.0022x over previous
"""Trainium2 Bass kernel for DyadicCrossAttention.

Sharding: 8 cores = 2 batches x 4 head-groups (2 heads of d=32 each, i.e. a
64-channel slice of HID=256). Each core computes its batch's q/k/v projections
for its 64 channels, both cross-attention directions for its 2 heads, and a
partial output projection Wo[:, ch] @ out[ch]. The host sums the 4 per-batch
partials and adds the constant bias vectors (bo + Wo @ bv, both independent of
the pixel, since softmax weights sum to 1).

Matmul operands are fp16 (fp32 PSUM accumulation). The softmax exp is the
bottleneck; it is split across TWO engines:
 - the scalar (ACT) engine runs activation(Exp, scale=16) at 1 col/cycle,
 - the vector (DVE) engine runs a pair of custom microprogrammed ops
   (EXP16A: clamped relative-minimax cubic of e^r, EXP16B: ^16 + scale) on a
   ~20% slice of the score tiles, at ~2.2x the ACT per-tile cost but in
   parallel with it. SCALE/16 is folded into Wq/bq host-side so PSUM scores
   are r = logit/16 in [-0.30, 0.425]; composite rel error ~1.4e-3.

Direction order: dir-2 (q2 k1 -> v1) runs FIRST because its keys come from
modal-1 (half the DMA bytes of modal-2), so the exp stream starts ~3us in
while x2 is still loading. Dir-1 follows and ends on the small 256-col block
to shorten the drain tail. Scores are transposed (keys on partitions);
softmax denominators ride along as ones-columns in the attn*V matmul, and
per-block epilogues (normalize + Wo projection) are deferred into the next
block so the in-order engines never interpose long work before the next
exp's input is ready.
"""

import sys

import numpy as np

sys.path.insert(0, "/opt/trn_rl_repo")

B, DIM1, DIM2, HID, HEADS, H, W, OUT = 2, 256, 512, 256, 8, 48, 48, 256
HD = HID // HEADS  # 32
SCALE = float(HD) ** -0.5
N = H * W  # 2304
NJ = N // 128  # 18 key chunks
NTILES = [(0, 256), (256, 512), (768, 512), (1280, 512), (1792, 512)]
NQUARTERS = [(0, 576), (576, 576), (1152, 576), (1728, 576)]

MM_MODE = "f16"        # "f16" or "bf16" matmul operand format
TRACE = False          # set by test.py for profiled runs
TRACE_KWARGS = {}
LAST_RESULTS = None    # BassKernelResults of the last run (for test.py)

# ---- custom DVE exp: out = exp(16*x) in two ops ----
# free minimax cubic for e^r on [-0.30, 0.425], normalized to P(0)=1
EXP_C1N = 0.999598597084961
EXP_C2N = 0.5050122210021222
EXP_C3N = 0.17626501373112596
EXP_PLO = 0.7408123653939622   # P(-0.30): clamp floor (== clamping r)
EXP_KAPPA = 0.9986888065713158  # c0^16

# j-steps of each processed block where the DVE "assists": head-0's exp
# stays on ACT (half-size instruction) and head-1's scores detour through a
# separate rvp PSUM tile that only the DVE custom-exp reads, so the stp
# double-buffer never waits on the slower DVE path.
# keyed by (dirpos, border_position); blocks are 18 j-steps each
# dir-2 (first) is PE-paced -- few assists, placed away from the j's where
# projection fillers hit the same rvp ring; dir-1 is ACT-paced -- many.
DVE_JS = {}
for _bp in range(5):
    DVE_JS[(0, _bp)] = () if _bp == 0 else ((5, 12) if _bp == 4 else (3, 8))
    DVE_JS[(1, _bp)] = () if _bp == 4 else (2, 4, 7, 9, 11, 14, 16)
KLOOK = 0      # k-ntile emission lookahead in key-chunks
EPI_OFF = (2, 4, 6)   # epilogue part due-offsets after block end

_CACHE = {}


def _register_exp16():
    from concourse import dve_ops
    from concourse.dve_spec import C0, C1, C2, One, Spec, Src0, lower, maxx, sq
    from concourse.dve_spec import _has_src1
    from concourse.dve_uop import DveOpSpec

    have = {op.name: op for op in dve_ops.OPS}
    if "EXP16A_ANT" in have and "EXP16B_ANT" in have:
        return have["EXP16A_ANT"], have["EXP16B_ANT"]

    def ref_a(in0, in1, s0, s1, imm2):
        x = in0.astype(np.float32)
        return ((s0 * x + s1) * (x * x) + (imm2 * x + 1.0)).astype(np.float32)

    def ref_b(in0, in1, s0, s1, imm2):
        p = np.maximum(in0.astype(np.float32), s0)
        for _ in range(4):
            p = (p * p).astype(np.float32)
        return (p * s1).astype(np.float32)

    spec_a = Spec(body=(Src0 * C0 + C1) * sq(Src0) + (Src0 * C2 + One),
                  reference=ref_a)
    spec_b = Spec(body=sq(sq(sq(sq(maxx(Src0, C0))))) * C1, reference=ref_b)

    out = []
    for name, spec in (("EXP16A_ANT", spec_a), ("EXP16B_ANT", spec_b)):
        row = max(dve_ops._SUB_OPCODE_FOR_NAME.values()) + 1
        assert row < 0x20
        shas = {}
        for ver in ("v3", "v4"):
            tmp = DveOpSpec(name=name, opcode=row, uops=lower(spec, ver=ver),
                            rd1_en=_has_src1(spec))
            shas[ver] = tmp.sha(ver)
        op = dve_ops.DveOp(name, spec, subdim=False, uops_sha=shas)
        dve_ops.OPS.append(op)
        dve_ops.CUSTOM_DVE_SPECS[name] = spec
        dve_ops._SUB_OPCODE_FOR_NAME[name] = row
        out.append(op)
    return out


def _split_multiwait(nc, mybir, limit=1):
    """Walrus rejects instructions carrying >limit semaphore waits; move the
    excess onto InstNoOp instructions inserted just before on the same engine."""
    for f in nc.m.functions:
        for bb in f.blocks:
            out = []
            changed = False
            for inst in bb.instructions:
                si = inst.sync_info
                if si is not None and len(si.on_wait) > limit:
                    waits = list(si.on_wait)
                    pre, keep = waits[:-limit], waits[-limit:]
                    for ci in range(0, len(pre), limit):
                        nop = mybir.InstNoOp(
                            name=f"{inst.name}-ws{ci}", ins=[], outs=[]
                        )
                        nop.engine = inst.engine
                        nop.sync_info = mybir.SyncInfo(
                            on_wait=pre[ci : ci + limit], on_update=[]
                        )
                        out.append(nop)
                    inst.sync_info = mybir.SyncInfo(
                        on_wait=keep, on_update=list(si.on_update)
                    )
                    changed = True
                out.append(inst)
            if changed:
                bb.instructions = out


def _build(mm_mode=None, reps=1):
    import concourse.bass as bass
    import concourse.tile as tile
    from concourse import mybir
    from concourse.library_overlay import lower_extended_insts

    if mm_mode is None:
        mm_mode = MM_MODE
    f32 = mybir.dt.float32
    md = {"bf16": mybir.dt.bfloat16, "f16": mybir.dt.float16}[mm_mode]
    AF = mybir.ActivationFunctionType
    op_a, op_b = _register_exp16()

    nc = bass.Bass()

    x1_d = nc.dram_tensor("x1", [128, 2, N], md, kind="ExternalInput")
    x2_d = nc.dram_tensor("x2", [128, 4, N], md, kind="ExternalInput")
    wqk1_d = nc.dram_tensor("wqk1", [128, 2, 128], md, kind="ExternalInput")
    wv1_d = nc.dram_tensor("wv1", [128, 2, 64], md, kind="ExternalInput")
    wqk2_d = nc.dram_tensor("wqk2", [128, 4, 128], md, kind="ExternalInput")
    wv2_d = nc.dram_tensor("wv2", [128, 4, 64], md, kind="ExternalInput")
    wo_d = nc.dram_tensor("wo", [32, 4, OUT], md, kind="ExternalInput")
    bqk1_d = nc.dram_tensor("bqk1", [128, 1], f32, kind="ExternalInput")
    bqk2_d = nc.dram_tensor("bqk2", [128, 1], f32, kind="ExternalInput")
    e33_d = nc.dram_tensor("e33", [33, 64], md, kind="ExternalInput")
    # partition-major output layout: [128 partitions, 2 row-halves, N]
    y1_d = nc.dram_tensor("y1", [128, 2, N], f32, kind="ExternalOutput")
    y2_d = nc.dram_tensor("y2", [128, 2, N], f32, kind="ExternalOutput")

    with tile.TileContext(nc) as tc, nc.allow_low_precision(
        reason="f16 matmul operands by design; fp32 psum accumulation"
    ):
        with tc.tile_pool(name="const", bufs=1) as const:
            # ---- resident tensors / DMAs ----
            # dir-2 runs first, so modal-1 (its key/value source) loads with
            # priority on the sync queue; modal-2 streams on the gpsimd
            # queue (its first chunk early: q2 block 0 gates the first
            # scores).
            wqk1 = const.tile([128, 2, 128], md)
            nc.sync.dma_start(wqk1[:], wqk1_d[:])
            # wqk2 + small early tensors ride the scalar-engine DGE queue so
            # they don't serialize the two x-streams (ACT is idle this early)
            wqk2 = const.tile([128, 4, 128], md)
            nc.scalar.dma_start(wqk2[:], wqk2_d[:])
            x1s = const.tile([128, 2, N], md)
            x2s = const.tile([128, 4, N], md)
            # prologue pieces split per channel-chunk so each projection
            # matmul starts as soon as its own contraction slice lands
            for c in range(2):
                nc.sync.dma_start(x1s[:, c, 0:256], x1_d[:, c, 0:256])
            # dir-2 starts on its 512-col block (cols 256-768): q2 of that
            # ntile is the only x2 piece in the critical DMA window
            for c0 in range(2):
                nc.gpsimd.dma_start(
                    x2s[:, 2 * c0 : 2 * c0 + 2, 256:768],
                    x2_d[:, 2 * c0 : 2 * c0 + 2, 256:768],
                )
            bqk1 = const.tile([128, 1], f32)
            nc.scalar.dma_start(bqk1[:], bqk1_d[:])
            bqk2 = const.tile([128, 1], f32)
            nc.scalar.dma_start(bqk2[:], bqk2_d[:])
            wv1 = const.tile([128, 2, 64], md)
            nc.scalar.dma_start(wv1[:], wv1_d[:])
            wv2 = const.tile([128, 4, 64], md)
            wo = const.tile([32, 4, OUT], md)
            e33 = const.tile([33, 64], md)
            nc.scalar.dma_start(e33[:], e33_d[:])
            # the DMA engines drain ~serially in issue order: x1's remaining
            # ntile chunks (dir-2's key sweep k1 <- x1 paces the first ~20us)
            # interleaved with x2's per-block q2 chunks just ahead of each
            # block's start; x2's tail (v2, q2 of the final small block)
            # lands long before dir-1 consumes it.
            nc.sync.dma_start(x1s[:, :, 256:768], x1_d[:, :, 256:768])
            nc.sync.dma_start(x1s[:, :, 768:1280], x1_d[:, :, 768:1280])
            nc.sync.dma_start(x2s[:, :, 768:1280], x2_d[:, :, 768:1280])
            nc.sync.dma_start(x1s[:, :, 1280:1792], x1_d[:, :, 1280:1792])
            nc.sync.dma_start(x1s[:, :, 1792:2304], x1_d[:, :, 1792:2304])
            nc.sync.dma_start(x2s[:, :, 1280:1792], x2_d[:, :, 1280:1792])
            nc.gpsimd.dma_start(wv2[:], wv2_d[:])
            nc.sync.dma_start(x2s[:, :, 1792:2304], x2_d[:, :, 1792:2304])
            nc.sync.dma_start(x2s[:, :, 0:256], x2_d[:, :, 0:256])
            nc.sync.dma_start(wo[:], wo_d[:])
            wo1a, wo1b = wo[:, 0, :], wo[:, 1, :]
            wo2a, wo2b = wo[:, 2, :], wo[:, 3, :]

            # projections (channel-major q/k; pixel-major v with ones columns)
            q1s = const.tile([64, N], md)
            k1s = const.tile([64, N], md)
            q2s = const.tile([64, N], md)
            k2s = const.tile([64, N], md)
            # per key-chunk j, 128 columns: [v_h0(32) | 1 | pad] [v_h1(32) | 1 | pad]
            v1T = const.tile([128, NJ, 128], md)
            v2T = const.tile([128, NJ, 128], md)
            for vt in (v1T, v2T):
                for ones_col in (vt[:, :, 32:33], vt[:, :, 96:97]):
                    nc.vector.memset(ones_col, 1.0)
            # reciprocal staging: rows 0 and 32 live, the rest stays zero
            rdens = []
            for _ri in range(2):
                rd = const.tile([64, 512], md, name=f"rden{_ri}")
                nc.vector.memset(rd[:], 0.0)
                rdens.append(rd)

            with (
                tc.tile_pool(name="stp", bufs=2, space="PSUM") as stp,
                tc.tile_pool(name="u0p", bufs=2, space="PSUM") as u0p,
                tc.tile_pool(name="rvp", bufs=1, space="PSUM") as rvp,
                tc.tile_pool(name="opp", bufs=1, space="PSUM") as opp,
                tc.tile_pool(name="ptp", bufs=6) as ptp,
                tc.tile_pool(name="midp", bufs=2) as midp,
                tc.tile_pool(name="sbs", bufs=3) as sbs,
            ):
                # ---- deferred projection emission ----
                qk_done = {(i, m): False
                           for i in range(len(NTILES)) for m in (1, 2)}
                v_done = {(m, j): False for m in (1, 2) for j in range(NJ)}

                def emit_qk_m(i, m, pool=None, k_first=False, defer_other=None):
                    if qk_done[(i, m)]:
                        return
                    qk_done[(i, m)] = True
                    n0, nt = NTILES[i]
                    wqk, xs, nch, qdst, kdst, bqk = (
                        (wqk1, x1s, 2, q1s, k1s, bqk1) if m == 1
                        else (wqk2, x2s, 4, q2s, k2s, bqk2)
                    )
                    pool = pool or opp
                    tag = "op" if pool is opp else "rv"
                    ps = pool.tile([128, 512], f32, tag=tag, name=f"qkps{i}_{m}")
                    for c in range(nch):
                        nc.tensor.matmul(
                            ps[:, :nt], wqk[:, c, :], xs[:, c, n0 : n0 + nt],
                            start=(c == 0), stop=(c == nch - 1),
                        )
                    copies = [
                        (qdst, ps[0:64, :nt], bqk[0:64, :]),
                        (kdst, ps[64:128, :nt], bqk[64:128, :]),
                    ]
                    if k_first:
                        copies.reverse()
                    if defer_other is not None:
                        # only the first (needed) half now, split per head so
                        # the first score matmul can start after 32 rows;
                        # queue the other half off the critical path
                        dst, src, b = copies[0]
                        nc.vector.tensor_scalar_add(
                            dst[0:32, n0 : n0 + nt], src[0:32], b[0:32]
                        )
                        nc.vector.tensor_scalar_add(
                            dst[32:64, n0 : n0 + nt], src[32:64], b[32:64]
                        )
                        dst, src, b = copies[1]
                        work_q.append((defer_other, (
                            lambda dst=dst, src=src, b=b:
                            nc.vector.tensor_scalar_add(dst[:, n0:n0 + nt], src, b)
                        )))
                        return
                    for dst, src, b in copies:
                        nc.vector.tensor_scalar_add(dst[:, n0 : n0 + nt], src, b)

                def emit_v(mod, j):
                    if v_done[(mod, j)]:
                        return
                    v_done[(mod, j)] = True
                    xs, wv, nch, vdst = (
                        (x1s, wv1, 2, v1T) if mod == 1 else (x2s, wv2, 4, v2T)
                    )
                    pvt = rvp.tile([128, 64], f32, tag="rv", name=f"pv{mod}_{j}")
                    for c in range(nch):
                        nc.tensor.matmul(
                            pvt[:], xs[:, c, j * 128 : (j + 1) * 128], wv[:, c, :],
                            start=(c == 0), stop=(c == nch - 1),
                        )
                    nc.vector.tensor_copy(vdst[:, j, 0:32], pvt[:, 0:32])
                    nc.vector.tensor_copy(vdst[:, j, 64:96], pvt[:, 32:64])

                pending_v = [(1, j) for j in range(NJ)] + [(2, j) for j in range(NJ)]
                pv_idx = [0]

                def trickle_v():
                    while pv_idx[0] < len(pending_v) and v_done[pending_v[pv_idx[0]]]:
                        pv_idx[0] += 1
                    if pv_idx[0] < len(pending_v):
                        mod, j = pending_v[pv_idx[0]]
                        emit_v(mod, j)

                # ---- prologue ----
                # unblock the first scores (dir 2: k1^T q2, first processed
                # block is ntile 1)
                work_q = []   # (ready_t, closure) FIFO; head runs when ready
                emit_qk_m(0, 1, pool=rvp, k_first=True, defer_other=2)
                emit_qk_m(1, 2, pool=opp, k_first=False, defer_other=3)

                # ---- attention: flat software-pipelined stream ----
                LAG = 2
                items = []
                for dirpos, (qs, ks, vT, woa, wob, ydst, vmod, border) in enumerate((
                    (q2s, k1s, v1T, wo2a, wo2b, y2_d, 1, (1, 2, 3, 4, 0)),
                    (q1s, k2s, v2T, wo1a, wo1b, y1_d, 2, (1, 2, 3, 4, 0)),
                ) * reps):
                    for bpos, bi in enumerate(border):
                        n0, nt = NTILES[bi]
                        dve_js = DVE_JS.get((dirpos % 2, bpos), ())
                        for j in range(NJ):
                            items.append((qs, ks, vT, woa, wob, ydst, vmod,
                                          bi, n0, nt, j, j in dve_js))

                blkc = [0]

                def mk_epi_parts(ut, rden, woa, wob, ydst, n0, nt):
                    """The per-block epilogue, split into three work-queue
                    pieces so no single pop clumps >1us on one engine. The
                    DVE chain is ordered rb0,o0,rb1,o1 so the first output
                    matmul can start after only half of it."""
                    u0t = ut[0:33, :]
                    u1t = ut[64:97, :]
                    o0 = sbs.tile([32, 512], md, tag="o0")
                    o1 = sbs.tile([32, 512], md, tag="o1")

                    def epi_a():
                        rbt = rvp.tile([64, 512], f32, tag="rv", name="rbt")
                        nc.tensor.matmul(rbt[:, :nt], e33[:], rden[0:33, :nt])
                        rb = sbs.tile([64, 512], f32, tag="rb")
                        nc.vector.tensor_copy(rb[:, :nt], rbt[:, :nt])
                        nc.vector.tensor_mul(o0[:, :nt], u0t[0:32, :nt], rb[0:32, :nt])
                        nc.vector.tensor_mul(o1[:, :nt], u1t[0:32, :nt], rb[32:64, :nt])

                    if nt <= 256:
                        # small block (incl. the drain-tail block): both
                        # output halves in one PSUM tile and one DMA
                        def epi_small():
                            opt = opp.tile([128, 2, 256], f32, tag="op")
                            for mt in range(2):
                                nc.tensor.matmul(
                                    opt[:, mt, :nt],
                                    woa[:, mt * 128 : (mt + 1) * 128],
                                    o0[:, :nt], start=True, stop=False,
                                )
                                nc.tensor.matmul(
                                    opt[:, mt, :nt],
                                    wob[:, mt * 128 : (mt + 1) * 128],
                                    o1[:, :nt], start=False, stop=True,
                                )
                            yt = sbs.tile([128, 2, 256], f32, tag="yt")
                            nc.vector.tensor_copy(yt[:, :, :nt], opt[:, :, :nt])
                            nc.sync.dma_start(
                                ydst[:, :, n0 : n0 + nt], yt[:, :, :nt]
                            )
                        def epi_nop():
                            pass
                        return epi_a, epi_small, epi_nop

                    def mk_epi_mt(mt):
                        def epi_mt():
                            opt = opp.tile([128, 512], f32, tag="op")
                            nc.tensor.matmul(
                                opt[:, :nt], woa[:, mt * 128 : (mt + 1) * 128],
                                o0[:, :nt], start=True, stop=False,
                            )
                            nc.tensor.matmul(
                                opt[:, :nt], wob[:, mt * 128 : (mt + 1) * 128],
                                o1[:, :nt], start=False, stop=True,
                            )
                            yt = sbs.tile([128, 512], f32, tag="yt")
                            nc.vector.tensor_copy(yt[:, :nt], opt[:, :nt])
                            nc.sync.dma_start(
                                ydst[:, mt, n0 : n0 + nt], yt[:, :nt]
                            )
                        return epi_mt

                    return epi_a, mk_epi_mt(0), mk_epi_mt(1)

                cur_u = {}    # "live" ut/rden for the consuming block
                pts = {}

                def emit_u(t2, it, t_now):
                    (qs, ks, vT, woa, wob, ydst, vmod, bi, n0, nt, jj, _dv) = it
                    emit_v(vmod, jj)
                    if jj == 0:
                        ut = u0p.tile([128, 512], f32, tag="u0", name="ut")
                        rden = rdens[blkc[0] % 2]
                        blkc[0] += 1
                        cur_u[(id(items), t2 - jj)] = (ut, rden)
                    ut, rden = cur_u[(id(items), t2 - jj)]
                    u0t = ut[0:33, :]
                    u1t = ut[64:97, :]
                    pt = pts.pop(t2)
                    nc.tensor.matmul(
                        u0t[:, :nt], vT[:, jj, 0:33], pt[:, 0, :nt],
                        start=(jj == 0), stop=(jj == NJ - 1),
                    )
                    nc.tensor.matmul(
                        u1t[:, :nt], vT[:, jj, 64:97], pt[:, 1, :nt],
                        start=(jj == 0), stop=(jj == NJ - 1),
                    )
                    if jj == NJ - 1:
                        del cur_u[(id(items), t2 - jj)]
                        nc.vector.reciprocal(rden[0:1, :nt], u0t[32:33, :nt])
                        nc.vector.reciprocal(rden[32:33, :nt], u1t[32:33, :nt])
                        ea, e0, e1 = mk_epi_parts(ut, rden, woa, wob, ydst,
                                                  n0, nt)
                        work_q.append((t_now + EPI_OFF[0], ea))
                        work_q.append((t_now + EPI_OFF[1], e0))
                        work_q.append((t_now + EPI_OFF[2], e1))

                t = 0
                n_items = len(items)
                next_u = [0]
                while (t < n_items or next_u[0] < n_items or work_q
                       or pv_idx[0] < len(pending_v)):
                    if t < n_items:
                        it = items[t]
                        (qs, ks, vT, woa, wob, ydst, vmod, bi, n0, nt, j,
                         dve_exp) = it
                        kmod = vmod                # k comes from the v modal
                        qmod = 3 - vmod
                        emit_qk_m(bi, qmod)
                        if j == 8 and t + NJ < n_items:
                            # pre-emit the next block's q projection mid-block
                            # so it's off the critical path at the boundary
                            nit = items[t + NJ - 8]
                            work_q.append(
                                (t + 1, (lambda nbi=nit[7], nqm=3 - nit[6]:
                                         emit_qk_m(nbi, nqm)))
                            )
                        # emit the k-projection ntile a few key-chunks ahead
                        # of its first consumer so the proj+copy chain is off
                        # the scores' critical path
                        last_col = 128 * min(j + 1 + KLOOK, NJ) - 1
                        for ni, (tn0, tnt) in enumerate(NTILES):
                            if tn0 <= last_col < tn0 + tnt:
                                if not qk_done[(ni, kmod)]:
                                    emit_qk_m(ni, kmod)
                                    if not qk_done[(ni, qmod)]:
                                        # other modal's projection of this
                                        # ntile: needed later, spread it out
                                        work_q.append(
                                            (t + 1, (lambda ni=ni, qmod=qmod:
                                                     emit_qk_m(ni, qmod)))
                                        )
                                break
                        st = stp.tile([128, 2, 512], f32, tag="st")
                        pt = ptp.tile([128, 2, 512], md, tag="pt")
                        nc.tensor.matmul(
                            st[:, 0, :nt],
                            ks[0:32, j * 128 : (j + 1) * 128],
                            qs[0:32, n0 : n0 + nt],
                        )
                        if dve_exp:
                            # head-1 scores into a decoupled PSUM slot; ACT
                            # only exps head 0 (half-size instruction)
                            rv_t = rvp.tile([128, 512], f32, tag="rv",
                                            name="dve_st")
                            nc.tensor.matmul(
                                rv_t[:, :nt],
                                ks[32:64, j * 128 : (j + 1) * 128],
                                qs[32:64, n0 : n0 + nt],
                            )
                            nc.scalar.activation(
                                pt[:, 0, :nt], st[:, 0, :nt], AF.Exp,
                                scale=16.0,
                            )
                            mid = midp.tile([128, 512], f32, tag="mid")
                            nc.vector._custom_dve(
                                op_a, out=mid[:, :nt], in0=rv_t[:, :nt],
                                s0=EXP_C3N, s1=EXP_C2N, imm2=EXP_C1N,
                            )
                            nc.vector._custom_dve(
                                op_b, out=pt[:, 1, :nt], in0=mid[:, :nt],
                                s0=EXP_PLO, s1=EXP_KAPPA,
                            )
                        else:
                            nc.tensor.matmul(
                                st[:, 1, :nt],
                                ks[32:64, j * 128 : (j + 1) * 128],
                                qs[32:64, n0 : n0 + nt],
                            )
                            nc.scalar.activation(
                                pt[:, :, :nt], st[:, :, :nt], AF.Exp, scale=16.0
                            )
                        pts[t] = pt
                    # attn*V consumption: strictly in tile order (the PSUM
                    # accumulation group's stop matmul must be emitted last),
                    # but per-tile due times let the slower DVE-exp tiles lag
                    # further behind, and spread the first block's tail into
                    # the next block's filler slots.
                    emitted_u = 0
                    while next_u[0] < min(t + 1, n_items) and emitted_u < 2:
                        t2 = next_u[0]
                        it2 = items[t2]
                        jj = it2[10]
                        due = t2 + LAG
                        if t2 < NJ and jj >= 13:
                            due = NJ + 2 + (jj - 13)
                        elif it2[11]:
                            due = t2 + LAG + 3
                        if due > t:
                            break
                        next_u[0] += 1
                        emitted_u += 1
                        emit_u(t2, it2, t)
                    if work_q and work_q[0][0] <= t:
                        work_q.pop(0)[1]()
                    elif pv_idx[0] < len(pending_v):
                        trickle_v()
                    t += 1

    _split_multiwait(nc, mybir, limit=1)
    lower_extended_insts(nc)
    return nc


def _get_nc():
    key = ("nc", MM_MODE)
    if key not in _CACHE:
        _CACHE[key] = _build()
    return _CACHE[key]


def kernel(
    modal1_feat, modal2_feat, Wq1, bq1, Wk1, bk1, Wv1, bv1,
    Wq2, bq2, Wk2, bk2, Wv2, bv2, Wo1, bo1, Wo2, bo2,
):
    global LAST_RESULTS
    from concourse.bass_utils import run_bass_kernel_spmd

    if MM_MODE == "bf16":
        import ml_dtypes
        md_np = ml_dtypes.bfloat16
    else:
        md_np = np.float16

    f = np.float32
    modal1_feat = np.asarray(modal1_feat, f)
    modal2_feat = np.asarray(modal2_feat, f)
    Wq1, bq1 = np.asarray(Wq1, f), np.asarray(bq1, f)
    Wk1, bk1 = np.asarray(Wk1, f), np.asarray(bk1, f)
    Wv1, bv1 = np.asarray(Wv1, f), np.asarray(bv1, f)
    Wq2, bq2 = np.asarray(Wq2, f), np.asarray(bq2, f)
    Wk2, bk2 = np.asarray(Wk2, f), np.asarray(bk2, f)
    Wv2, bv2 = np.asarray(Wv2, f), np.asarray(bv2, f)
    Wo1, bo1 = np.asarray(Wo1, f), np.asarray(bo1, f)
    Wo2, bo2 = np.asarray(Wo2, f), np.asarray(bo2, f)

    # fold SCALE/16 into the q projections: PSUM scores become logit/16
    qs = SCALE / 16.0
    Wq1s, bq1s = Wq1 * qs, bq1 * qs
    Wq2s, bq2s = Wq2 * qs, bq2 * qs

    e33 = np.zeros((33, 64), f)
    e33[0, 0:32] = 1.0
    e33[32, 32:64] = 1.0

    def cvt(a):
        return np.ascontiguousarray(np.asarray(a, md_np))

    def pack(a, nch):
        # [nch*128, cols] -> partition-major [128, nch, cols]
        return a.reshape(nch, 128, -1).transpose(1, 0, 2)

    in_maps = []
    for core in range(8):
        b, hg = core // 4, core % 4
        ch = slice(hg * 64, hg * 64 + 64)
        cha = slice(hg * 64, hg * 64 + 32)
        chb = slice(hg * 64 + 32, hg * 64 + 64)
        in_maps.append({
            "x1": cvt(pack(modal1_feat[b].reshape(DIM1, N), 2)),
            "x2": cvt(pack(modal2_feat[b].reshape(DIM2, N), 4)),
            "wqk1": cvt(pack(np.concatenate([Wq1s[ch].T, Wk1[ch].T], axis=1), 2)),
            "wv1": cvt(pack(Wv1[ch].T, 2)),
            "wqk2": cvt(pack(np.concatenate([Wq2s[ch].T, Wk2[ch].T], axis=1), 4)),
            "wv2": cvt(pack(Wv2[ch].T, 4)),
            "wo": cvt(np.stack([Wo1[:, cha].T, Wo1[:, chb].T,
                                Wo2[:, cha].T, Wo2[:, chb].T], axis=1)),
            "bqk1": np.ascontiguousarray(
                np.concatenate([bq1s[ch], bk1[ch]])[:, None]),
            "bqk2": np.ascontiguousarray(
                np.concatenate([bq2s[ch], bk2[ch]])[:, None]),
            "e33": cvt(e33),
        })

    nc = _get_nc()
    res = run_bass_kernel_spmd(
        nc, in_maps, core_ids=list(range(8)), trace=TRACE, **TRACE_KWARGS
    )
    LAST_RESULTS = res

    out1 = np.zeros((B, OUT, N), f)
    out2 = np.zeros((B, OUT, N), f)
    for core in range(8):
        b = core // 4
        out1[b] += res.results[core]["y1"].transpose(1, 0, 2).reshape(OUT, N)
        out2[b] += res.results[core]["y2"].transpose(1, 0, 2).reshape(OUT, N)
    # constant (per-pixel-independent) bias terms: bo + Wo @ bv
    out1 += (bo1 + Wo1 @ bv2)[None, :, None]
    out2 += (bo2 + Wo2 @ bv1)[None, :, None]
    return (
        out1.reshape(B, OUT, H, W),
        out2.reshape(B, OUT, H, W),
    )
